# revision 2
# baseline (speedup 1.0000x reference)
"""V8: partition-major x layout + tuned warmup.

On top of V7:
- x shards are stored PARTITION-MAJOR ([128, C, D] instead of
  [C, 128, D]): each partition's group slice is one contiguous
  gs*D-byte segment, so a group DMA is 128 large descriptors instead
  of gs*128 x 1-2KB strided ones - better SDMA efficiency, ~6x less
  descriptor-ring pressure, cheaper descriptor generation.
- Warmup is sized to bridge the PE from its first free slot to
  c2-ready (~4us): the clock ramp needs CONTINUOUS busy, and any idle
  gap resets it to 1.2 GHz.

V7 notes that still apply:

From the V5/V6 traces:
- Metadata goes FIRST on the sync HWDGE ring: ring FIFO guarantees it
  drains before the x-group flood (a second-ring DMA contends for the
  16 SDMA engines and lands ~5us late).
- GpSimd partition_all_reduce takes ~6us and running it during the
  stream cost ~25% of DMA bandwidth (SWDGE/SDMA port contention), so
  the Z normalizer is computed by two tiny PE matmuls placed AFTER the
  last stream matmul: the PE is at full clock by then and the ~0.7us
  chain overlaps the PSUM copies' semaphore latency.
- The PE clock ramps 1.2->2.4 GHz after ~3us of CONTINUOUS busy and
  any idle gap resets it, so the PE queue is warmup + stream matmuls
  with nothing interleaved.
- Tail copies read two separate PSUM tiles (parallel ACT/DVE copies +
  parallel stores on the two HWDGE rings).
"""

import numpy as np
import ml_dtypes

import concourse.bass as bass
import concourse.tile as tile
from concourse import bacc, bass_isa, mybir
from concourse.bass_utils import run_bass_kernel_spmd
from concourse.vector_clock import ScopedClock


class _LeanTileContext(tile.TileContext):
    """TileContext with a lighter kernel epilogue (see V2)."""

    def _drain_and_barrier(self, tick_clock, wait_clock):
        drain_inst = self.nc.sync.drain()
        wait_clock.add_sem_waits(
            drain_inst.ins, ScopedClock({None: tick_clock.global_clock})
        )
        self.nc.all_engine_barrier()
        popped = self.nc._tile_sem_poison_stack.pop()
        assert popped is self._sem_poison
        self.nc.clear_and_free_semaphores(list(self.sems.allocated().values()))

B, T, D = 16, 2048, 1024
NCORES = 8
F32 = mybir.dt.float32
BF16 = mybir.dt.bfloat16
FP8 = mybir.dt.float8e3          # e3m4: 4 mantissa bits, matmul at bf16 rate

NP_BF16 = ml_dtypes.bfloat16
NP_FP8 = ml_dtypes.float8_e3m4

GSZ = 6               # max chunks per DMA
WARMUP_MMS = 8
BIG = 1.0e9           # t-sentinel for "element not owned by this row"
FP8_MASS_MIN = 0.35   # fp8 tier gets at least this much coeff^2 mass
FP8_MASS_MAX = 0.45   # ... and at most this much (while shrinking bf16 tier)


def _plan(c, end_taper):
    sizes = []
    rem = c
    end = []
    if end_taper:
        for s in (1, 2):
            if rem <= s:
                break
            end.append(s)
            rem -= s
        end = end[::-1]
    while rem > 0:
        s = min(GSZ, rem)
        sizes.append(s)
        rem -= s
    return sizes + end


def _build_program(tiers):
    """tiers: tuple of (dtype_key, nchunks), in global chunk order."""
    nc = bacc.Bacc(
        "TRN2", target_bir_lowering=False, debug=False, num_devices=NCORES
    )
    DTS = {"bf16": BF16, "fp8": FP8}
    C = sum(ct for _, ct in tiers)
    # merged metadata (f32, per partition):
    #   [0:16) w2d | [16:16+C) wc | [16+C:16+C+C*B) tcr | [...:+B) lens
    M = 16 + C + C * B + B

    xcs = []
    for ti, (key, ct) in enumerate(tiers):
        xcs.append(
            nc.dram_tensor(f"xc{ti}", [128, ct, D], DTS[key], kind="ExternalInput").ap()
        )
    meta = nc.dram_tensor("meta", [128, M], F32, kind="ExternalInput").ap()
    out = nc.dram_tensor("out", [B, D], F32, kind="ExternalOutput").ap()

    # global group plan: (tier_idx, local_k0, gs, global_k0)
    groups = []
    goff = 0
    for ti, (key, ct) in enumerate(tiers):
        for s_k0, s in _iter_plan(ct, end_taper=(ti == len(tiers) - 1)):
            groups.append((ti, s_k0, s, goff + s_k0))
        goff += ct
    from collections import Counter
    tag_counts = Counter((tiers[ti][0], gs) for ti, _, gs, _ in groups)

    with _LeanTileContext(nc) as tc:
        with (
            tc.tile_pool(name="consts", bufs=1) as consts,
            tc.tile_pool(name="xin", bufs=1) as xpool,
            tc.tile_pool(name="outs", bufs=1) as opool,
            tc.tile_pool(name="psum", bufs=1, space="PSUM") as pacc,
            tc.tile_pool(name="psumz", bufs=1, space="PSUM") as pz,
        ):
            # --- sync ring: metadata FIRST (ring FIFO -> it drains ahead
            # of the x flood), then every x group upfront ---
            mt = consts.tile([128, M], F32)
            nc.sync.dma_start(out=mt, in_=meta)

            xts = []
            for ti, k0, gs, gk0 in groups:
                key = tiers[ti][0]
                xt = xpool.tile([128, gs, D], DTS[key], name="xt",
                                tag=f"xt_{key}_{gs}", bufs=tag_counts[(key, gs)])
                nc.sync.dma_start(
                    out=xt, in_=xcs[ti][:, k0 : k0 + gs, :],
                )
                xts.append(xt)

            def mview(start, dims):
                return bass.AP(tensor=mt.tensor, offset=mt.offset + start,
                               ap=[mt.ap[0]] + dims)

            w2d = mview(0, [[1, 16]])
            wc = mview(16, [[1, C]])
            tcr = mview(16 + C, [[B, C], [1, B]])
            lens_b = mview(16 + C + C * B, [[0, C], [1, B]])

            # --- DVE constants (no meta dependency: issue first) ---
            ones128 = consts.tile([128, 1], F32)
            nc.vector.memset(ones128, 1.0)
            ones16 = consts.tile([1, B], F32)
            nc.vector.memset(ones16, 1.0)
            warm_rhs = consts.tile([128, 512], BF16)
            nc.vector.memset(warm_rhs.bitcast(F32), 0.0)
            warm_lhs = consts.tile([128, 16], BF16)
            nc.vector.memset(warm_lhs.bitcast(F32), 0.0)

            # --- Z inputs: exp+accum on the scalar engine ---
            e2d = consts.tile([128, 16], F32)
            zpart = consts.tile([128, 1], F32)
            nc.scalar.activation(
                out=e2d, in_=w2d, func=mybir.ActivationFunctionType.Exp,
                accum_out=zpart,
            )
            ec = consts.tile([128, C], F32)
            nc.scalar.activation(
                out=ec, in_=wc, func=mybir.ActivationFunctionType.Exp,
            )
            mask = consts.tile([128, C, B], F32)
            nc.vector.tensor_tensor(
                out=mask, in0=tcr, in1=lens_b, op=mybir.AluOpType.is_lt,
            )
            ec_b = bass.AP(
                tensor=ec.tensor, offset=ec.offset,
                ap=[ec.ap[0], ec.ap[1], [0, B]],
            )
            c2 = consts.tile([128, C, B], BF16)
            nc.vector.tensor_tensor(
                out=c2, in0=mask, in1=ec_b, op=mybir.AluOpType.mult,
            )

            # --- PE queue: warmup then the stream, nothing else ---
            pwarm = pz.tile([16, 512], F32, name="pwarm", tag="pwarm")
            for _ in range(WARMUP_MMS):
                nc.tensor.matmul(pwarm, lhsT=warm_lhs, rhs=warm_rhs,
                                 start=True, stop=True)

            # --- main streaming loop (two PSUM tiles, one per D half) ---
            psf0 = pacc.tile([B, 512], F32, name="psf0", tag="ps0")
            psf1 = pacc.tile([B, 512], F32, name="psf1", tag="ps1")
            ps = [psf0, psf1]
            for gi, (ti, k0, gs, gk0) in enumerate(groups):
                xt = xts[gi]
                for j in range(gs):
                    k = gk0 + j
                    for dh in range(2):
                        nc.tensor.matmul(
                            ps[dh], lhsT=c2[:, k, :],
                            rhs=xt[:, j, dh * 512 : (dh + 1) * 512],
                            start=(k == 0), stop=(k == C - 1),
                        )

            # --- Z chain on the now-warm, now-idle PE ---
            psz1 = pz.tile([1, 1], F32, name="psz1", tag="psz1")
            nc.tensor.matmul(psz1, lhsT=zpart, rhs=ones128, start=True, stop=True)
            zsb = consts.tile([1, 1], F32)
            nc.vector.tensor_scalar_mul(zsb, psz1, 1.0)
            psz16 = pz.tile([B, 1], F32, name="psz16", tag="psz16")
            nc.tensor.matmul(psz16, lhsT=ones16, rhs=zsb, start=True, stop=True)
            rz = consts.tile([B, 1], F32)
            nc.vector.reciprocal(rz, psz16)

            # --- tail: scaled PSUM->SBUF copies on DVE + ACT in parallel,
            # stores on the two HWDGE rings in parallel ---
            ot0 = opool.tile([B, 512], F32, name="ot0", tag="ot0")
            ot1 = opool.tile([B, 512], F32, name="ot1", tag="ot1")
            nc.vector.tensor_scalar(
                out=ot1, in0=psf1, scalar1=rz,
                scalar2=None, op0=mybir.AluOpType.mult,
            )
            nc.scalar.mul(ot0, psf0, rz)
            nc.scalar.dma_start(out=out[:, 0:512], in_=ot0)
            nc.sync.dma_start(out=out[:, 512:1024], in_=ot1)

    nc.compile()
    return nc


def _iter_plan(c, end_taper):
    sizes = _plan(c, end_taper)
    k0 = 0
    for s in sizes:
        yield k0, s
        k0 += s


_cache = {}


def _get_program(tiers):
    if tiers not in _cache:
        _cache[tiers] = _build_program(tiers)
    return _cache[tiers]


def kernel(input, lengths, weights):
    input = np.asarray(input, dtype=np.float32)
    lengths_np = np.asarray(lengths).astype(np.int64)
    weights = np.asarray(weights, dtype=np.float32)

    lens_clip = np.clip(lengths_np, 0, T)
    total_rows = int(lens_clip.sum())

    # --- tier assignment: bottom-coefficient timesteps -> fp8, with the
    # mass cut adapted so the bf16 tier fits one chunk per core ---
    c = np.exp(weights - weights.max())
    mult = (np.arange(T)[None, :] < lens_clip[:, None]).sum(0)  # [T]
    mass = c * c * mult
    order = np.argsort(c, kind="stable")
    cum = np.cumsum(mass[order])
    cum_rows = np.cumsum(mult[order])
    tot = max(cum[-1], 1e-30)
    ncut = int(np.searchsorted(cum, FP8_MASS_MIN * tot))
    while (
        ncut < T
        and total_rows - (cum_rows[ncut - 1] if ncut else 0) > 128 * NCORES
        and cum[ncut] <= FP8_MASS_MAX * tot
    ):
        ncut += 1
    is_fp8_t = np.zeros(T, dtype=bool)
    is_fp8_t[order[:ncut]] = True

    b_flat = np.repeat(np.arange(B, dtype=np.int64), lens_clip)
    t_flat = np.concatenate(
        [np.arange(n, dtype=np.int64) for n in lens_clip]
    ) if total_rows else np.zeros(0, dtype=np.int64)
    fp8_rows = is_fp8_t[t_flat] if total_rows else np.zeros(0, dtype=bool)

    def pack(bsel, tsel):
        n = len(bsel)
        ct = -(-n // (128 * NCORES))
        cap = ct * 128 * NCORES
        bp = np.concatenate([bsel, np.full(cap - n, -1, dtype=np.int64)])
        tp = np.concatenate([tsel, np.zeros(cap - n, dtype=np.int64)])
        return ct, bp.reshape(NCORES, ct, 128), tp.reshape(NCORES, ct, 128)

    c16, b16, t16 = pack(b_flat[~fp8_rows], t_flat[~fp8_rows])
    c8, b8, t8 = pack(b_flat[fp8_rows], t_flat[fp8_rows])
    tiers = tuple(
        (key, ct) for key, ct in (("bf16", c16), ("fp8", c8)) if ct > 0
    )
    if not tiers:  # degenerate: no live rows at all
        tiers = (("bf16", 1),)
        c16 = 1
        b16 = np.full((NCORES, 1, 128), -1, dtype=np.int64)
        t16 = np.zeros((NCORES, 1, 128), dtype=np.int64)

    nc = _get_program(tiers)

    C = sum(ct for _, ct in tiers)
    M = 16 + C + C * B + B
    w2d = weights.reshape(128, 16)
    lens_f = lengths_np.astype(np.float32)
    flat2d = input.reshape(B * T, D)
    rb = np.arange(B)

    in_maps = []
    for cidx in range(NCORES):
        per_tier = []
        if c16 > 0:
            per_tier.append((b16[cidx], t16[cidx], NP_BF16))
        if c8 > 0:
            per_tier.append((b8[cidx], t8[cidx], NP_FP8))

        m = {}
        bs_all = []
        ts_all = []
        for ti, (bs, ts, npdt) in enumerate(per_tier):
            xc = flat2d[np.maximum(bs, 0) * T + ts]  # [ct, 128, D]
            m[f"xc{ti}"] = np.ascontiguousarray(
                xc.transpose(1, 0, 2)
            ).astype(npdt)
            bs_all.append(bs)
            ts_all.append(ts)
        bs = np.concatenate(bs_all, axis=0)          # [C, 128]
        ts = np.concatenate(ts_all, axis=0)

        wcm = weights[ts].T                           # [128, C]
        tcrm = np.where(
            bs[:, :, None] == rb[None, None, :],
            ts[:, :, None].astype(np.float32), BIG,
        ).transpose(1, 0, 2)                          # [128, C, B]

        metam = np.empty((128, M), dtype=np.float32)
        metam[:, 0:16] = w2d
        metam[:, 16:16 + C] = wcm
        metam[:, 16 + C:16 + C + C * B] = tcrm.reshape(128, C * B)
        metam[:, 16 + C + C * B:] = lens_f[None, :]
        m["meta"] = metam
        in_maps.append(m)

    res = run_bass_kernel_spmd(nc, in_maps, list(range(NCORES)))
    out = np.zeros((B, D), dtype=np.float32)
    for cidx in range(NCORES):
        out += res.results[cidx]["out"]
    return out.astype(np.float32)


# revision 3
# speedup vs baseline: 1.0882x; 1.0882x over previous
"""V12: V11 + the DoubleRow-e4m3 bottom tier from V10.

On top of V8:
- The wc/tcr/lens metadata rides as fp16 (t-coords and lengths are
  integers <= 2048, exactly representable; the sentinel becomes 6e4):
  the metadata DMA that gates the coefficient build halves in size.
- Main group sizes are emitted smallest-first so the ramping PE gets
  its first big-group completion semaphore sooner.
- Warmup 6 (the PE queue, not the data, was gating stream start).

V8 notes that still apply:

On top of V7:
- x shards are stored PARTITION-MAJOR ([128, C, D] instead of
  [C, 128, D]): each partition's group slice is one contiguous
  gs*D-byte segment, so a group DMA is 128 large descriptors instead
  of gs*128 x 1-2KB strided ones - better SDMA efficiency, ~6x less
  descriptor-ring pressure, cheaper descriptor generation.
- Warmup is sized to bridge the PE from its first free slot to
  c2-ready (~4us): the clock ramp needs CONTINUOUS busy, and any idle
  gap resets it to 1.2 GHz.

V7 notes that still apply:

From the V5/V6 traces:
- Metadata goes FIRST on the sync HWDGE ring: ring FIFO guarantees it
  drains before the x-group flood (a second-ring DMA contends for the
  16 SDMA engines and lands ~5us late).
- GpSimd partition_all_reduce takes ~6us and running it during the
  stream cost ~25% of DMA bandwidth (SWDGE/SDMA port contention), so
  the Z normalizer is computed by two tiny PE matmuls placed AFTER the
  last stream matmul: the PE is at full clock by then and the ~0.7us
  chain overlaps the PSUM copies' semaphore latency.
- The PE clock ramps 1.2->2.4 GHz after ~3us of CONTINUOUS busy and
  any idle gap resets it, so the PE queue is warmup + stream matmuls
  with nothing interleaved.
- Tail copies read two separate PSUM tiles (parallel ACT/DVE copies +
  parallel stores on the two HWDGE rings).
"""

import numpy as np
import ml_dtypes

import concourse.bass as bass
import concourse.tile as tile
from concourse import bacc, bass_isa, mybir
from concourse.bass_utils import run_bass_kernel_spmd
from concourse.vector_clock import ScopedClock


class _LeanTileContext(tile.TileContext):
    """TileContext with a lighter kernel epilogue (see V2)."""

    def _drain_and_barrier(self, tick_clock, wait_clock):
        drain_inst = self.nc.sync.drain()
        wait_clock.add_sem_waits(
            drain_inst.ins, ScopedClock({None: tick_clock.global_clock})
        )
        self.nc.all_engine_barrier()
        popped = self.nc._tile_sem_poison_stack.pop()
        assert popped is self._sem_poison
        self.nc.clear_and_free_semaphores(list(self.sems.allocated().values()))

B, T, D = 16, 2048, 1024
NCORES = 8
F32 = mybir.dt.float32
F16 = mybir.dt.float16
BF16 = mybir.dt.bfloat16
FP8 = mybir.dt.float8e3          # e3m4: 4 mantissa bits, matmul at bf16 rate
FP8DR = mybir.dt.float8e4        # e4m3: DoubleRow-capable

NP_BF16 = ml_dtypes.bfloat16
NP_FP8 = ml_dtypes.float8_e3m4
NP_FP8DR = ml_dtypes.float8_e4m3

GSZ = 6               # max chunks per DMA
WARMUP_MMS = 6
BIG = 6.0e4           # t-sentinel (fits fp16) for "element not owned by this row"
FP8_MASS_MIN = 0.35   # fp8 tiers get at least this much coeff^2 mass
FP8_MASS_MAX = 0.45   # ... and at most this much (while shrinking bf16 tier)
DR_MASS = 0.05        # bottom band that rides DoubleRow e4m3


def _plan(c, end_taper):
    sizes = []
    rem = c
    end = []
    if end_taper:
        for s in (1, 2):
            if rem <= s:
                break
            end.append(s)
            rem -= s
        end = end[::-1]
    while rem > 0:
        s = min(GSZ, rem)
        sizes.append(s)
        rem -= s
    sizes.sort()
    return sizes + end


def _plan_even(c):
    assert c % 2 == 0
    sizes = []
    rem = c
    end = [2] if rem > 2 else []
    rem -= 2 * len(end)
    while rem > 0:
        s = min(GSZ, rem)
        sizes.append(s)
        rem -= s
    sizes.sort()
    return sizes + end


def _build_program(tiers):
    """tiers: tuple of (dtype_key, nchunks), in global chunk order."""
    nc = bacc.Bacc(
        "TRN2", target_bir_lowering=False, debug=False, num_devices=NCORES
    )
    DTS = {"bf16": BF16, "fp8": FP8, "dr8": FP8DR}
    C = sum(ct for _, ct in tiers)
    # merged metadata, per partition: 16 f32 (w2d), then an fp16 section
    # [0:C) wc | [C:C+C*B) tcr | [...:+B) lens
    L = C + C * B + B
    M = 16 + (L + 1) // 2

    xcs = []
    for ti, (key, ct) in enumerate(tiers):
        xcs.append(
            nc.dram_tensor(f"xc{ti}", [128, ct, D], DTS[key], kind="ExternalInput").ap()
        )
    meta = nc.dram_tensor("meta", [128, M], F32, kind="ExternalInput").ap()
    out = nc.dram_tensor("out", [B, D], F32, kind="ExternalOutput").ap()

    # global group plan: (tier_idx, local_k0, gs, global_k0)
    groups = []
    goff = 0
    for ti, (key, ct) in enumerate(tiers):
        if key == "dr8":
            sizes = _plan_even(ct)
        else:
            sizes = _plan(ct, end_taper=(ti == len(tiers) - 1))
        k0 = 0
        for s in sizes:
            groups.append((ti, k0, s, goff + k0))
            k0 += s
        goff += ct
    from collections import Counter
    tag_counts = Counter((tiers[ti][0], gs) for ti, _, gs, _ in groups)

    with _LeanTileContext(nc) as tc:
        with (
            tc.tile_pool(name="consts", bufs=1) as consts,
            tc.tile_pool(name="xin", bufs=1) as xpool,
            tc.tile_pool(name="outs", bufs=1) as opool,
            tc.tile_pool(name="psum", bufs=1, space="PSUM") as pacc,
            tc.tile_pool(name="psumz", bufs=1, space="PSUM") as pz,
        ):
            # --- sync ring: metadata FIRST (ring FIFO -> it drains ahead
            # of the x flood), then every x group upfront ---
            mt = consts.tile([128, M], F32)
            nc.sync.dma_start(out=mt, in_=meta)

            xts = []
            for ti, k0, gs, gk0 in groups:
                key = tiers[ti][0]
                xt = xpool.tile([128, gs, D], DTS[key], name="xt",
                                tag=f"xt_{key}_{gs}", bufs=tag_counts[(key, gs)])
                nc.sync.dma_start(
                    out=xt, in_=xcs[ti][:, k0 : k0 + gs, :],
                )
                xts.append(xt)

            def mview(start, dims):
                return bass.AP(tensor=mt.tensor, offset=mt.offset + start,
                               ap=[mt.ap[0]] + dims)

            w2d = mview(0, [[1, 16]])
            mt16 = mt.bitcast(F16)

            def mview16(start, dims):
                return bass.AP(tensor=mt16.tensor, offset=mt16.offset + 32 + start,
                               ap=[mt16.ap[0]] + dims)

            wc = mview16(0, [[1, C]])
            tcr = mview16(C, [[B, C], [1, B]])
            lens_b = mview16(C + C * B, [[0, C], [1, B]])

            # --- DVE constants (no meta dependency: issue first) ---
            ones128 = consts.tile([128, 1], F32)
            nc.vector.memset(ones128, 1.0)
            ones16 = consts.tile([1, B], F32)
            nc.vector.memset(ones16, 1.0)
            warm_rhs = consts.tile([128, 512], BF16)
            nc.vector.memset(warm_rhs.bitcast(F32), 0.0)
            warm_lhs = consts.tile([128, 16], BF16)
            nc.vector.memset(warm_lhs.bitcast(F32), 0.0)

            # --- Z inputs: exp+accum on the scalar engine ---
            e2d = consts.tile([128, 16], F32)
            zpart = consts.tile([128, 1], F32)
            nc.scalar.activation(
                out=e2d, in_=w2d, func=mybir.ActivationFunctionType.Exp,
                accum_out=zpart,
            )
            ec = consts.tile([128, C], F32)
            nc.scalar.activation(
                out=ec, in_=wc, func=mybir.ActivationFunctionType.Exp,
            )
            mask = consts.tile([128, C, B], F32)
            nc.vector.tensor_tensor(
                out=mask, in0=tcr, in1=lens_b, op=mybir.AluOpType.is_lt,
            )
            ec_b = bass.AP(
                tensor=ec.tensor, offset=ec.offset,
                ap=[ec.ap[0], ec.ap[1], [0, B]],
            )
            c2 = consts.tile([128, C, B], BF16)
            nc.vector.tensor_tensor(
                out=c2, in0=mask, in1=ec_b, op=mybir.AluOpType.mult,
            )
            cdr = next((ct for key, ct in tiers if key == "dr8"), 0)
            if cdr:
                c2dr = consts.tile([128, cdr, B], FP8DR)
                nc.vector.tensor_scalar_mul(c2dr, c2[:, C - cdr:, :], 1.0)

            # --- PE queue: warmup then the stream, nothing else ---
            pwarm = pz.tile([16, 512], F32, name="pwarm", tag="pwarm")
            for _ in range(WARMUP_MMS):
                nc.tensor.matmul(pwarm, lhsT=warm_lhs, rhs=warm_rhs,
                                 start=True, stop=True)

            # --- main streaming loop (two PSUM tiles, one per D half) ---
            psf0 = pacc.tile([B, 512], F32, name="psf0", tag="ps0")
            psf1 = pacc.tile([B, 512], F32, name="psf1", tag="ps1")
            ps = [psf0, psf1]
            for gi, (ti, k0, gs, gk0) in enumerate(groups):
                xt = xts[gi]
                if tiers[ti][0] == "dr8":
                    for j in range(0, gs, 2):
                        k = gk0 + j
                        kd = k - (C - cdr)
                        for dh in range(2):
                            nc.tensor.matmul(
                                ps[dh], lhsT=c2dr[:, kd : kd + 2, :],
                                rhs=xt[:, j : j + 2, dh * 512 : (dh + 1) * 512],
                                start=(k == 0), stop=(k + 1 == C - 1),
                                perf_mode=mybir.MatmulPerfMode.DoubleRow,
                            )
                else:
                    for j in range(gs):
                        k = gk0 + j
                        for dh in range(2):
                            nc.tensor.matmul(
                                ps[dh], lhsT=c2[:, k, :],
                                rhs=xt[:, j, dh * 512 : (dh + 1) * 512],
                                start=(k == 0), stop=(k == C - 1),
                            )

            # --- Z chain on the now-warm, now-idle PE ---
            psz1 = pz.tile([1, 1], F32, name="psz1", tag="psz1")
            nc.tensor.matmul(psz1, lhsT=zpart, rhs=ones128, start=True, stop=True)
            zsb = consts.tile([1, 1], F32)
            nc.vector.tensor_scalar_mul(zsb, psz1, 1.0)
            psz16 = pz.tile([B, 1], F32, name="psz16", tag="psz16")
            nc.tensor.matmul(psz16, lhsT=ones16, rhs=zsb, start=True, stop=True)
            rz = consts.tile([B, 1], F32)
            nc.vector.reciprocal(rz, psz16)

            # --- tail: scaled PSUM->SBUF copies on DVE + ACT in parallel,
            # stores on the two HWDGE rings in parallel ---
            ot0 = opool.tile([B, 512], F32, name="ot0", tag="ot0")
            ot1 = opool.tile([B, 512], F32, name="ot1", tag="ot1")
            nc.vector.tensor_scalar(
                out=ot1, in0=psf1, scalar1=rz,
                scalar2=None, op0=mybir.AluOpType.mult,
            )
            nc.scalar.mul(ot0, psf0, rz)
            nc.scalar.dma_start(out=out[:, 0:512], in_=ot0)
            nc.sync.dma_start(out=out[:, 512:1024], in_=ot1)

    nc.compile()
    return nc


def _iter_plan(c, end_taper):
    sizes = _plan(c, end_taper)
    k0 = 0
    for s in sizes:
        yield k0, s
        k0 += s


_cache = {}


def _get_program(tiers):
    if tiers not in _cache:
        _cache[tiers] = _build_program(tiers)
    return _cache[tiers]


def kernel(input, lengths, weights):
    input = np.asarray(input, dtype=np.float32)
    lengths_np = np.asarray(lengths).astype(np.int64)
    weights = np.asarray(weights, dtype=np.float32)

    lens_clip = np.clip(lengths_np, 0, T)
    total_rows = int(lens_clip.sum())

    # --- tier assignment: bottom-coefficient timesteps -> fp8, with the
    # mass cut adapted so the bf16 tier fits one chunk per core ---
    c = np.exp(weights - weights.max())
    mult = (np.arange(T)[None, :] < lens_clip[:, None]).sum(0)  # [T]
    mass = c * c * mult
    order = np.argsort(c, kind="stable")
    cum = np.cumsum(mass[order])
    cum_rows = np.cumsum(mult[order])
    tot = max(cum[-1], 1e-30)
    ncut = int(np.searchsorted(cum, FP8_MASS_MIN * tot))
    while (
        ncut < T
        and total_rows - (cum_rows[ncut - 1] if ncut else 0) > 128 * NCORES
        and cum[ncut] <= FP8_MASS_MAX * tot
    ):
        ncut += 1
    ndr = min(int(np.searchsorted(cum, DR_MASS * tot)), ncut)
    tier_t = np.zeros(T, dtype=np.int64)       # 0=bf16, 1=e3m4, 2=dr-e4m3
    tier_t[order[:ncut]] = 1
    tier_t[order[:ndr]] = 2

    b_flat = np.repeat(np.arange(B, dtype=np.int64), lens_clip)
    t_flat = np.concatenate(
        [np.arange(n, dtype=np.int64) for n in lens_clip]
    ) if total_rows else np.zeros(0, dtype=np.int64)
    row_tier = tier_t[t_flat] if total_rows else np.zeros(0, dtype=np.int64)

    def pack(bsel, tsel, even=False):
        n = len(bsel)
        ct = -(-n // (128 * NCORES))
        if even and ct % 2:
            ct += 1
        cap = ct * 128 * NCORES
        bp = np.concatenate([bsel, np.full(cap - n, -1, dtype=np.int64)])
        tp = np.concatenate([tsel, np.zeros(cap - n, dtype=np.int64)])
        return ct, bp.reshape(NCORES, ct, 128), tp.reshape(NCORES, ct, 128)

    c16, b16, t16 = pack(b_flat[row_tier == 0], t_flat[row_tier == 0])
    c8, b8, t8 = pack(b_flat[row_tier == 1], t_flat[row_tier == 1])
    cdr, bdr, tdr = pack(b_flat[row_tier == 2], t_flat[row_tier == 2], even=True)
    tiers = tuple(
        (key, ct)
        for key, ct in (("bf16", c16), ("fp8", c8), ("dr8", cdr))
        if ct > 0
    )
    if not tiers:  # degenerate: no live rows at all
        tiers = (("bf16", 1),)
        c16 = 1
        b16 = np.full((NCORES, 1, 128), -1, dtype=np.int64)
        t16 = np.zeros((NCORES, 1, 128), dtype=np.int64)

    nc = _get_program(tiers)

    C = sum(ct for _, ct in tiers)
    L = C + C * B + B
    M = 16 + (L + 1) // 2
    w2d = weights.reshape(128, 16)
    lens_f = lengths_np.astype(np.float32)
    flat2d = input.reshape(B * T, D)
    rb = np.arange(B)

    in_maps = []
    for cidx in range(NCORES):
        per_tier = []
        if c16 > 0:
            per_tier.append((b16[cidx], t16[cidx], NP_BF16))
        if c8 > 0:
            per_tier.append((b8[cidx], t8[cidx], NP_FP8))
        if cdr > 0:
            per_tier.append((bdr[cidx], tdr[cidx], NP_FP8DR))

        m = {}
        bs_all = []
        ts_all = []
        for ti, (bs, ts, npdt) in enumerate(per_tier):
            xc = flat2d[np.maximum(bs, 0) * T + ts]  # [ct, 128, D]
            m[f"xc{ti}"] = np.ascontiguousarray(
                xc.transpose(1, 0, 2)
            ).astype(npdt)
            bs_all.append(bs)
            ts_all.append(ts)
        bs = np.concatenate(bs_all, axis=0)          # [C, 128]
        ts = np.concatenate(ts_all, axis=0)

        wcm = weights[ts].T                           # [128, C]
        tcrm = np.where(
            bs[:, :, None] == rb[None, None, :],
            ts[:, :, None].astype(np.float16), np.float16(BIG),
        ).transpose(1, 0, 2)                          # [128, C, B]

        half = np.zeros((128, 2 * ((L + 1) // 2)), dtype=np.float16)
        half[:, 0:C] = wcm.astype(np.float16)
        half[:, C:C + C * B] = tcrm.reshape(128, C * B)
        half[:, C + C * B:L] = lens_f.astype(np.float16)[None, :]
        metam = np.empty((128, M), dtype=np.float32)
        metam[:, 0:16] = w2d
        metam[:, 16:] = half.view(np.float32)
        m["meta"] = metam
        in_maps.append(m)

    res = run_bass_kernel_spmd(nc, in_maps, list(range(NCORES)))
    out = np.zeros((B, D), dtype=np.float32)
    for cidx in range(NCORES):
        out += res.results[cidx]["out"]
    return out.astype(np.float32)
